# revision 8
# baseline (speedup 1.0000x reference)
"""DVQ bottleneck kernel for Trainium2, data-parallel over 8 NeuronCores.

Problem (hardcoded): h [8, 4096, 1024] f32, codebooks [4, 16, 256] f32.
Per token t and slice s: ids[t,s] = argmin_k ||ze_ts - c_sk||^2,
z = gathered codebook rows, ids packed base-16, vq loss = 1.25 * sum of
min squared distances / (B*N*d).

Sharding: 32768 tokens split 4096/core across 8 cores; codebooks replicated.

Per-core dataflow (tokens on partitions, 128/sub-block):
  DMA h tile [128, 1024] -> PE transpose to hT [d, t] (PSUM) -> ACT/DVE copy
  to SBUF -> PE matmul scores[t, (s,k)] = -2*ze.c (contract d, 2 chunks) ->
  DVE: +|c|^2, reduce_min, first-index argmin via is_equal/iota trick ->
  onehot [t,(s,k)] -> PE transpose -> PE matmul with block-diag codebook
  -> z [t, 1024] (PSUM) -> copy -> DMA out.
  Loss: ACT square+accum for sum(h^2), min-scores accumulated on DVE,
  final partition reduce via ones-matmul.
"""

import os
import numpy as np

import concourse.bass as bass
import concourse.bacc as bacc
import concourse.mybir as mybir
from concourse.tile import TileContext
from concourse.bass_utils import run_bass_kernel_spmd
from concourse.masks import make_identity

F32 = mybir.dt.float32
I32 = mybir.dt.int32

B, N, D = 8, 4096, 1024
S, K, d = 4, 16, 256
NCORES = 8
P = 128
NCHUNK = D // P  # 8
BETA = 0.25


def build_nc(t_core: int) -> bass.Bass:
    nsub = t_core // P
    nmac = nsub // 4
    assert nmac * 4 == nsub

    nc = bacc.Bacc()

    h_d = nc.declare_dram_parameter("h", [t_core, D], F32, isOutput=False)
    cts_d = nc.declare_dram_parameter("cts", [P, NCHUNK, K], F32, isOutput=False)
    cbd_d = nc.declare_dram_parameter("cbd", [S * K, D], F32, isOutput=False)
    cnorm_d = nc.declare_dram_parameter("cnorm4", [P, 4 * S * K], F32, isOutput=False)
    iotad_d = nc.declare_dram_parameter("iotad4", [P, 4 * S * K], F32, isOutput=False)
    offs_d = nc.declare_dram_parameter("offs4", [P, 4 * S], F32, isOutput=False)
    z_d = nc.declare_dram_parameter("z", [t_core, D], F32, isOutput=True)
    ids_d = nc.declare_dram_parameter("ids", [nsub, P], I32, isOutput=True)
    part_d = nc.declare_dram_parameter("partial", [1, 1], F32, isOutput=True)

    with TileContext(nc) as tc:
        with (
            tc.tile_pool(name="consts", bufs=1) as consts,
            tc.tile_pool(name="hpool", bufs=3) as hpool,
            tc.tile_pool(name="hTsb", bufs=2) as hTsb_pool,
            tc.tile_pool(name="zsb", bufs=2) as zsb_pool,
            tc.tile_pool(name="small", bufs=2) as small,
            tc.tile_pool(name="accp", bufs=1) as accp,
            tc.tile_pool(name="ps_hT", bufs=2, space="PSUM") as ps_hT,
            tc.tile_pool(name="ps_z", bufs=1, space="PSUM") as ps_z,
            tc.tile_pool(name="ps_sc", bufs=1, space="PSUM") as ps_sc,
            tc.tile_pool(name="ps_oh", bufs=1, space="PSUM") as ps_oh,
        ):
            # ---- constants ----
            identity = consts.tile([P, P], F32)
            make_identity(nc, identity)
            cts = consts.tile([P, NCHUNK, K], F32)
            nc.sync.dma_start(out=cts, in_=cts_d[:, :, :])
            cbd = consts.tile([S * K, D], F32)
            nc.sync.dma_start(out=cbd, in_=cbd_d[:, :])
            cnorm4 = consts.tile([P, 4 * S * K], F32)
            nc.sync.dma_start(out=cnorm4, in_=cnorm_d[:, :])
            iotad4 = consts.tile([P, 4 * S * K], F32)
            nc.sync.dma_start(out=iotad4, in_=iotad_d[:, :])
            offs4 = consts.tile([P, 4 * S], F32)
            nc.sync.dma_start(out=offs4, in_=offs_d[:, :])
            ones = consts.tile([P, 1], F32)
            nc.vector.memset(ones, 1.0)

            # ---- accumulators ----
            acc16 = accp.tile([P, 4 * S], F32)
            nc.vector.memset(acc16, 0.0)
            hsq = accp.tile([P, nsub], F32)
            idsf = accp.tile([P, nsub], F32)
            sqscratch = accp.tile([P, D], F32)

            for m in range(nmac):
                sc_ps = ps_sc.tile([P, 4, S * K], F32, tag="sc")
                for j in range(4):
                    t0 = (4 * m + j) * P
                    col = 4 * m + j
                    h_t = hpool.tile([P, D], F32, tag="h")
                    nc.sync.dma_start(out=h_t, in_=h_d[t0 : t0 + P, :])
                    # sum of squares for the loss (ACT, fused reduce)
                    nc.scalar.activation(
                        out=sqscratch,
                        in_=h_t,
                        func=mybir.ActivationFunctionType.Square,
                        accum_out=hsq[:, col : col + 1],
                    )
                    # transpose h tile chunkwise: hT[d, t]
                    hT_ps = ps_hT.tile([P, D], F32, tag="hT")
                    for c in range(NCHUNK):
                        nc.tensor.transpose(
                            hT_ps[:, c * P : (c + 1) * P],
                            h_t[:, c * P : (c + 1) * P],
                            identity,
                        )
                    hT_s = hTsb_pool.tile([P, D], F32, tag="hTs")
                    if j % 2 == 0:
                        nc.scalar.copy(out=hT_s, in_=hT_ps)
                    else:
                        nc.vector.tensor_copy(out=hT_s, in_=hT_ps)
                    # scores: -2 * ze . c per slice (accumulate 2 chunks of d)
                    for s in range(S):
                        nc.tensor.matmul(
                            sc_ps[:, j, s * K : (s + 1) * K],
                            lhsT=hT_s[:, (2 * s) * P : (2 * s + 1) * P],
                            rhs=cts[:, 2 * s, :],
                            start=True,
                            stop=False,
                        )
                        nc.tensor.matmul(
                            sc_ps[:, j, s * K : (s + 1) * K],
                            lhsT=hT_s[:, (2 * s + 1) * P : (2 * s + 2) * P],
                            rhs=cts[:, 2 * s + 1, :],
                            start=False,
                            stop=True,
                        )

                # ---- argmin over k for 512 tokens at once ----
                sc_sb = small.tile([P, 4 * S * K], F32, tag="scsb")
                nc.vector.tensor_tensor(
                    sc_sb, sc_ps.rearrange("p a b -> p (a b)"), cnorm4,
                    mybir.AluOpType.add,
                )
                sc3 = sc_sb.rearrange("p (g k) -> p g k", k=K)
                minv = small.tile([P, 4 * S], F32, tag="minv")
                nc.vector.tensor_reduce(
                    minv, sc3, axis=mybir.AxisListType.X, op=mybir.AluOpType.min
                )
                mask = small.tile([P, 4 * S * K], F32, tag="mask")
                nc.vector.tensor_tensor(
                    mask, sc3, minv[:, :, None].to_broadcast((P, 4 * S, K)),
                    mybir.AluOpType.is_equal,
                )
                t4 = small.tile([P, 4 * S * K], F32, tag="t4")
                nc.vector.tensor_tensor(t4, mask, iotad4, mybir.AluOpType.mult)
                dmax = small.tile([P, 4 * S], F32, tag="dmax")
                nc.vector.tensor_reduce(
                    dmax,
                    t4.rearrange("p (g k) -> p g k", k=K),
                    axis=mybir.AxisListType.X,
                    op=mybir.AluOpType.max,
                )
                # packed ids: 65535 - sum_s dmax * 16^s   (dmax = 15 - id)
                pk = small.tile([P, 4 * S], F32, tag="pk")
                nc.vector.tensor_tensor(pk, dmax, offs4, mybir.AluOpType.mult)
                pneg = small.tile([P, 4], F32, tag="pneg")
                nc.vector.tensor_reduce(
                    pneg,
                    pk.rearrange("p (a s) -> p a s", s=S),
                    axis=mybir.AxisListType.X,
                    op=mybir.AluOpType.add,
                )
                nc.vector.tensor_scalar(
                    idsf[:, 4 * m : 4 * m + 4], pneg, -1.0, 65535.0,
                    op0=mybir.AluOpType.mult, op1=mybir.AluOpType.add,
                )
                # clean one-hot (single 1 even on ties): iotad == dmax
                onehotT = small.tile([P, 4 * S * K], F32, tag="onehotT")
                nc.vector.tensor_tensor(
                    onehotT,
                    iotad4.rearrange("p (g k) -> p g k", k=K),
                    dmax[:, :, None].to_broadcast((P, 4 * S, K)),
                    mybir.AluOpType.is_equal,
                )
                # loss accumulation
                nc.vector.tensor_tensor(acc16, acc16, minv, mybir.AluOpType.add)

                # ---- gather z via one-hot matmul ----
                oh_ps = ps_oh.tile([S * K, 4, P], F32, tag="oh")
                for j in range(4):
                    nc.tensor.transpose(
                        oh_ps[:, j, :],
                        onehotT[:, j * S * K : (j + 1) * S * K],
                        identity,
                    )
                oh_sb = small.tile([S * K, 4, P], F32, tag="ohsb")
                nc.scalar.copy(out=oh_sb, in_=oh_ps)
                for j in range(4):
                    t0 = (4 * m + j) * P
                    z_ps = ps_z.tile([P, D], F32, tag="z")
                    nc.tensor.matmul(
                        z_ps[:, : D // 2], lhsT=oh_sb[:, j, :], rhs=cbd[:, : D // 2],
                        start=True, stop=True,
                    )
                    nc.tensor.matmul(
                        z_ps[:, D // 2 :], lhsT=oh_sb[:, j, :], rhs=cbd[:, D // 2 :],
                        start=True, stop=True,
                    )
                    z_s = zsb_pool.tile([P, D], F32, tag="zs")
                    if j % 2 == 0:
                        nc.vector.tensor_copy(out=z_s, in_=z_ps)
                    else:
                        nc.scalar.copy(out=z_s, in_=z_ps)
                    nc.sync.dma_start(out=z_d[t0 : t0 + P, :], in_=z_s)

            # ---- epilogue: ids out ----
            ids_ps = ps_sc.tile([nsub, P], F32, tag="sc")
            nc.tensor.transpose(ids_ps, idsf, identity)
            ids_int = small.tile([nsub, P], I32, tag="idsint")
            nc.vector.tensor_copy(out=ids_int, in_=ids_ps)
            nc.sync.dma_start(out=ids_d[:, :], in_=ids_int)

            # ---- epilogue: loss partial ----
            r1 = small.tile([P, 1], F32, tag="r1")
            nc.vector.tensor_reduce(
                r1, acc16, axis=mybir.AxisListType.X, op=mybir.AluOpType.add
            )
            r2 = small.tile([P, 1], F32, tag="r2")
            nc.vector.tensor_reduce(
                r2, hsq, axis=mybir.AxisListType.X, op=mybir.AluOpType.add
            )
            rt = small.tile([P, 1], F32, tag="rt")
            nc.vector.tensor_tensor(rt, r1, r2, mybir.AluOpType.add)
            part_ps = ps_oh.tile([1, 1], F32, tag="oh")
            nc.tensor.matmul(part_ps, lhsT=ones, rhs=rt, start=True, stop=True)
            part_sb = small.tile([1, 1], F32, tag="partsb")
            nc.vector.tensor_copy(out=part_sb, in_=part_ps)
            nc.sync.dma_start(out=part_d[:, :], in_=part_sb)

    nc.finalize()
    return nc


def make_consts(cb: np.ndarray) -> dict[str, np.ndarray]:
    cb = np.ascontiguousarray(cb, dtype=np.float32)
    # cts[dp, c, k] = -2 * cb[c // 2, k, (c % 2) * 128 + dp]
    cb_r = cb.reshape(S, K, 2, P)  # [s, k, half, dp]
    cts = np.ascontiguousarray(-2.0 * cb_r.transpose(3, 0, 2, 1).reshape(P, NCHUNK, K))
    cbd = np.zeros((S * K, D), dtype=np.float32)
    for s in range(S):
        cbd[s * K : (s + 1) * K, s * d : (s + 1) * d] = cb[s]
    cnorm1 = np.square(cb).sum(-1).reshape(-1)  # [S*K]
    cnorm4 = np.tile(cnorm1, (P, 4))
    iotad1 = np.tile((15.0 - np.arange(K)).astype(np.float32), 4 * S)
    iotad4 = np.tile(iotad1, (P, 1))
    offs1 = np.tile((16.0 ** np.arange(S)).astype(np.float32), 4)
    offs4 = np.tile(offs1, (P, 1))
    return {
        "cts": cts.astype(np.float32),
        "cbd": cbd,
        "cnorm4": cnorm4.astype(np.float32),
        "iotad4": iotad4.astype(np.float32),
        "offs4": offs4.astype(np.float32),
    }


_NC_CACHE: dict[int, bass.Bass] = {}
LAST_RESULTS = None


def _get_nc(t_core: int) -> bass.Bass:
    if t_core not in _NC_CACHE:
        _NC_CACHE[t_core] = build_nc(t_core)
    return _NC_CACHE[t_core]


def kernel(h: np.ndarray, codebooks: np.ndarray):
    h = np.ascontiguousarray(h, dtype=np.float32)
    cb = np.ascontiguousarray(codebooks, dtype=np.float32)
    t_core = (B * N) // NCORES
    nc = _get_nc(t_core)
    consts = make_consts(cb)
    hf = h.reshape(B * N, D)
    in_maps = []
    for c in range(NCORES):
        m = {"h": hf[c * t_core : (c + 1) * t_core]}
        m.update(consts)
        in_maps.append(m)
    global LAST_RESULTS
    kr = run_bass_kernel_spmd(nc, in_maps, list(range(NCORES)))
    LAST_RESULTS = kr
    res = kr.results
    z = np.concatenate([res[c]["z"] for c in range(NCORES)], axis=0).reshape(B, N, D)
    ids = np.concatenate(
        [res[c]["ids"].reshape(-1) for c in range(NCORES)]
    ).reshape(B, N).astype(np.int32)
    total = np.sum([np.float64(res[c]["partial"][0, 0]) for c in range(NCORES)])
    vq_total = np.float32((1.0 + BETA) * total / (B * N * d))
    return z, ids, vq_total


# revision 15
# speedup vs baseline: 1.3020x; 1.3020x over previous
"""DVQ bottleneck kernel for Trainium2, data-parallel over 8 NeuronCores.

Problem (hardcoded): h [8, 4096, 1024] f32, codebooks [4, 16, 256] f32.
Per token t and slice s: ids[t,s] = argmin_k ||ze_ts - c_sk||^2,
z = gathered codebook rows, ids packed base-16, vq loss = 1.25 * sum of
min squared distances / (B*N*d).

Sharding: 32768 tokens split 4096/core across 8 cores; codebooks replicated.

Per-core dataflow (tokens on partitions, 128/sub-block):
  DMA h tile [128, 1024] -> PE transpose to hT [d, t] (PSUM) -> ACT/DVE copy
  to SBUF -> PE matmul scores[t, (s,k)] = -2*ze.c (contract d, 2 chunks) ->
  DVE: +|c|^2, reduce_min, first-index argmin via is_equal/iota trick ->
  onehot [t,(s,k)] -> PE transpose -> PE matmul with block-diag codebook
  -> z [t, 1024] (PSUM) -> copy -> DMA out.
  Loss: ACT square+accum for sum(h^2), min-scores accumulated on DVE,
  final partition reduce via ones-matmul.
"""

import os
import numpy as np

import concourse.bass as bass
import concourse.bacc as bacc
import concourse.mybir as mybir
from concourse.tile import TileContext
from concourse.bass_utils import run_bass_kernel_spmd
from concourse.masks import make_identity

F32 = mybir.dt.float32
BF16 = mybir.dt.bfloat16
I32 = mybir.dt.int32

B, N, D = 8, 4096, 1024
S, K, d = 4, 16, 256
NCORES = 8
P = 128
NCHUNK = D // P  # 8
BETA = 0.25


def build_nc(t_core: int) -> bass.Bass:
    nsub = t_core // P
    nmac = nsub // 4
    assert nmac * 4 == nsub

    nc = bacc.Bacc()

    h_d = nc.declare_dram_parameter("h", [t_core, D], F32, isOutput=False)
    cts_d = nc.declare_dram_parameter("cts", [P, NCHUNK, K], F32, isOutput=False)
    cbd1_d = nc.declare_dram_parameter("cbd1", [S * K, D], BF16, isOutput=False)
    cbd2_d = nc.declare_dram_parameter("cbd2", [S * K, D], BF16, isOutput=False)
    cnorm_d = nc.declare_dram_parameter("cnorm4", [P, 4 * S * K], F32, isOutput=False)
    iotad_d = nc.declare_dram_parameter("iotad4", [P, 4 * S * K], F32, isOutput=False)
    offs_d = nc.declare_dram_parameter("offs4", [P, 4 * S], F32, isOutput=False)
    z_d = nc.declare_dram_parameter("z", [t_core, D], F32, isOutput=True)
    ids_d = nc.declare_dram_parameter("ids", [nsub, P], I32, isOutput=True)
    part_d = nc.declare_dram_parameter("partial", [1, 1], F32, isOutput=True)

    with TileContext(nc) as tc:
        with (
            tc.tile_pool(name="consts", bufs=1) as consts,
            tc.tile_pool(name="hpool", bufs=3) as hpool,
            tc.tile_pool(name="hTsb", bufs=2) as hTsb_pool,
            tc.tile_pool(name="zsb", bufs=2) as zsb_pool,
            tc.tile_pool(name="small", bufs=2) as small,
            tc.tile_pool(name="accp", bufs=1) as accp,
            tc.tile_pool(name="ps_hT", bufs=2, space="PSUM") as ps_hT,
            tc.tile_pool(name="ps_z", bufs=1, space="PSUM") as ps_z,
            tc.tile_pool(name="ps_sc", bufs=1, space="PSUM") as ps_sc,
            tc.tile_pool(name="ps_oh", bufs=1, space="PSUM") as ps_oh,
        ):
            # ---- constants ----
            identity = consts.tile([P, P], F32)
            make_identity(nc, identity)
            cts = consts.tile([P, NCHUNK, K], F32)
            nc.sync.dma_start(out=cts, in_=cts_d[:, :, :])
            cbd1 = consts.tile([S * K, D], BF16)
            nc.sync.dma_start(out=cbd1, in_=cbd1_d[:, :])
            cbd2 = consts.tile([S * K, D], BF16)
            nc.sync.dma_start(out=cbd2, in_=cbd2_d[:, :])
            identity_bf = consts.tile([P, P], BF16)
            nc.vector.tensor_copy(out=identity_bf, in_=identity)
            cnorm4 = consts.tile([P, 4 * S * K], F32)
            nc.sync.dma_start(out=cnorm4, in_=cnorm_d[:, :])
            iotad4 = consts.tile([P, 4 * S * K], F32)
            nc.sync.dma_start(out=iotad4, in_=iotad_d[:, :])
            offs4 = consts.tile([P, 4 * S], F32)
            nc.sync.dma_start(out=offs4, in_=offs_d[:, :])
            ones = consts.tile([P, 1], F32)
            nc.vector.memset(ones, 1.0)

            # ---- accumulators ----
            acc16 = accp.tile([P, 4 * S], F32)
            nc.vector.memset(acc16, 0.0)
            hsq = accp.tile([P, nsub], F32)
            idsf = accp.tile([P, nsub], F32)
            sqscratch = accp.tile([P, D], F32)

            for m in range(nmac):
                sc_ps = ps_sc.tile([P, 4, S * K], F32, tag="sc")
                for j in range(4):
                    t0 = (4 * m + j) * P
                    col = 4 * m + j
                    h_t = hpool.tile([P, D], F32, tag="h")
                    nc.sync.dma_start(out=h_t, in_=h_d[t0 : t0 + P, :])
                    # sum of squares for the loss (ACT, fused reduce)
                    nc.scalar.activation(
                        out=sqscratch,
                        in_=h_t,
                        func=mybir.ActivationFunctionType.Square,
                        accum_out=hsq[:, col : col + 1],
                    )
                    # transpose h tile chunkwise: hT[d, t]
                    hT_ps = ps_hT.tile([P, D], F32, tag="hT")
                    for c in range(NCHUNK):
                        nc.tensor.transpose(
                            hT_ps[:, c * P : (c + 1) * P],
                            h_t[:, c * P : (c + 1) * P],
                            identity,
                        )
                    hT_s = hTsb_pool.tile([P, D], F32, tag="hTs")
                    if j % 2 == 0:
                        nc.scalar.copy(out=hT_s, in_=hT_ps)
                    else:
                        nc.vector.tensor_copy(out=hT_s, in_=hT_ps)
                    # scores: -2 * ze . c per slice (accumulate 2 chunks of d)
                    for s in range(S):
                        nc.tensor.matmul(
                            sc_ps[:, j, s * K : (s + 1) * K],
                            lhsT=hT_s[:, (2 * s) * P : (2 * s + 1) * P],
                            rhs=cts[:, 2 * s, :],
                            start=True,
                            stop=False,
                        )
                        nc.tensor.matmul(
                            sc_ps[:, j, s * K : (s + 1) * K],
                            lhsT=hT_s[:, (2 * s + 1) * P : (2 * s + 2) * P],
                            rhs=cts[:, 2 * s + 1, :],
                            start=False,
                            stop=True,
                        )

                # ---- argmin over k for 512 tokens at once ----
                sc_sb = small.tile([P, 4 * S * K], F32, tag="scsb")
                nc.vector.tensor_tensor(
                    sc_sb, sc_ps.rearrange("p a b -> p (a b)"), cnorm4,
                    mybir.AluOpType.add,
                )
                sc3 = sc_sb.rearrange("p (g k) -> p g k", k=K)
                minv = small.tile([P, 4 * S], F32, tag="minv")
                nc.vector.tensor_reduce(
                    minv, sc3, axis=mybir.AxisListType.X, op=mybir.AluOpType.min
                )
                mask = small.tile([P, 4 * S * K], F32, tag="mask")
                nc.vector.tensor_tensor(
                    mask, sc3, minv[:, :, None].to_broadcast((P, 4 * S, K)),
                    mybir.AluOpType.is_equal,
                )
                t4 = small.tile([P, 4 * S * K], F32, tag="t4")
                nc.vector.tensor_tensor(t4, mask, iotad4, mybir.AluOpType.mult)
                dmax = small.tile([P, 4 * S], F32, tag="dmax")
                nc.vector.tensor_reduce(
                    dmax,
                    t4.rearrange("p (g k) -> p g k", k=K),
                    axis=mybir.AxisListType.X,
                    op=mybir.AluOpType.max,
                )
                # packed ids: 65535 - sum_s dmax * 16^s   (dmax = 15 - id)
                pk = small.tile([P, 4 * S], F32, tag="pk")
                nc.vector.tensor_tensor(pk, dmax, offs4, mybir.AluOpType.mult)
                pneg = small.tile([P, 4], F32, tag="pneg")
                nc.vector.tensor_reduce(
                    pneg,
                    pk.rearrange("p (a s) -> p a s", s=S),
                    axis=mybir.AxisListType.X,
                    op=mybir.AluOpType.add,
                )
                nc.vector.tensor_scalar(
                    idsf[:, 4 * m : 4 * m + 4], pneg, -1.0, 65535.0,
                    op0=mybir.AluOpType.mult, op1=mybir.AluOpType.add,
                )
                # clean one-hot (single 1 even on ties): iotad == dmax
                onehotT = small.tile([P, 4 * S * K], BF16, tag="onehotT")
                nc.vector.tensor_tensor(
                    onehotT,
                    iotad4.rearrange("p (g k) -> p g k", k=K),
                    dmax[:, :, None].to_broadcast((P, 4 * S, K)),
                    mybir.AluOpType.is_equal,
                )
                # loss accumulation
                nc.vector.tensor_tensor(acc16, acc16, minv, mybir.AluOpType.add)

                # ---- gather z via one-hot matmul (bf16, exact to 2^-17) ----
                oh_ps = ps_oh.tile([S * K, 4, P], BF16, tag="oh")
                for j in range(4):
                    nc.tensor.transpose(
                        oh_ps[:, j, :],
                        onehotT[:, j * S * K : (j + 1) * S * K],
                        identity_bf,
                    )
                oh_sb = small.tile([S * K, 4, P], BF16, tag="ohsb")
                nc.scalar.copy(out=oh_sb, in_=oh_ps)
                for j in range(4):
                    t0 = (4 * m + j) * P
                    z_ps = ps_z.tile([P, D], F32, tag="z")
                    for half in (0, 1):
                        sl = slice(half * (D // 2), (half + 1) * (D // 2))
                        nc.tensor.matmul(
                            z_ps[:, sl], lhsT=oh_sb[:, j, :], rhs=cbd1[:, sl],
                            start=True, stop=False,
                        )
                        nc.tensor.matmul(
                            z_ps[:, sl], lhsT=oh_sb[:, j, :], rhs=cbd2[:, sl],
                            start=False, stop=True,
                        )
                    z_s = zsb_pool.tile([P, D], F32, tag="zs")
                    if j % 2 == 0:
                        nc.vector.tensor_copy(out=z_s, in_=z_ps)
                    else:
                        nc.scalar.copy(out=z_s, in_=z_ps)
                    nc.sync.dma_start(out=z_d[t0 : t0 + P, :], in_=z_s)

            # ---- epilogue: ids out ----
            ids_ps = ps_sc.tile([nsub, P], F32, tag="sc")
            nc.tensor.transpose(ids_ps, idsf, identity)
            ids_int = small.tile([nsub, P], I32, tag="idsint")
            nc.vector.tensor_copy(out=ids_int, in_=ids_ps)
            nc.sync.dma_start(out=ids_d[:, :], in_=ids_int)

            # ---- epilogue: loss partial ----
            r1 = small.tile([P, 1], F32, tag="r1")
            nc.vector.tensor_reduce(
                r1, acc16, axis=mybir.AxisListType.X, op=mybir.AluOpType.add
            )
            r2 = small.tile([P, 1], F32, tag="r2")
            nc.vector.tensor_reduce(
                r2, hsq, axis=mybir.AxisListType.X, op=mybir.AluOpType.add
            )
            rt = small.tile([P, 1], F32, tag="rt")
            nc.vector.tensor_tensor(rt, r1, r2, mybir.AluOpType.add)
            part_ps = ps_oh.tile([1, 1], F32, tag="oh")
            nc.tensor.matmul(part_ps, lhsT=ones, rhs=rt, start=True, stop=True)
            part_sb = small.tile([1, 1], F32, tag="partsb")
            nc.vector.tensor_copy(out=part_sb, in_=part_ps)
            nc.sync.dma_start(out=part_d[:, :], in_=part_sb)

    nc.finalize()
    return nc


def make_consts(cb: np.ndarray) -> dict[str, np.ndarray]:
    cb = np.ascontiguousarray(cb, dtype=np.float32)
    # cts[dp, c, k] = -2 * cb[c // 2, k, (c % 2) * 128 + dp]
    cb_r = cb.reshape(S, K, 2, P)  # [s, k, half, dp]
    cts = np.ascontiguousarray(-2.0 * cb_r.transpose(3, 0, 2, 1).reshape(P, NCHUNK, K))
    cbd = np.zeros((S * K, D), dtype=np.float32)
    for s in range(S):
        cbd[s * K : (s + 1) * K, s * d : (s + 1) * d] = cb[s]
    import ml_dtypes
    cbd1 = cbd.astype(ml_dtypes.bfloat16)
    cbd2 = (cbd - cbd1.astype(np.float32)).astype(ml_dtypes.bfloat16)
    cnorm1 = np.square(cb).sum(-1).reshape(-1)  # [S*K]
    cnorm4 = np.tile(cnorm1, (P, 4))
    iotad1 = np.tile((15.0 - np.arange(K)).astype(np.float32), 4 * S)
    iotad4 = np.tile(iotad1, (P, 1))
    offs1 = np.tile((16.0 ** np.arange(S)).astype(np.float32), 4)
    offs4 = np.tile(offs1, (P, 1))
    return {
        "cts": cts.astype(np.float32),
        "cbd1": cbd1,
        "cbd2": cbd2,
        "cnorm4": cnorm4.astype(np.float32),
        "iotad4": iotad4.astype(np.float32),
        "offs4": offs4.astype(np.float32),
    }


_NC_CACHE: dict[int, bass.Bass] = {}
LAST_RESULTS = None


def _get_nc(t_core: int) -> bass.Bass:
    if t_core not in _NC_CACHE:
        _NC_CACHE[t_core] = build_nc(t_core)
    return _NC_CACHE[t_core]


def kernel(h: np.ndarray, codebooks: np.ndarray):
    h = np.ascontiguousarray(h, dtype=np.float32)
    cb = np.ascontiguousarray(codebooks, dtype=np.float32)
    t_core = (B * N) // NCORES
    nc = _get_nc(t_core)
    consts = make_consts(cb)
    hf = h.reshape(B * N, D)
    in_maps = []
    for c in range(NCORES):
        m = {"h": hf[c * t_core : (c + 1) * t_core]}
        m.update(consts)
        in_maps.append(m)
    global LAST_RESULTS
    kr = run_bass_kernel_spmd(nc, in_maps, list(range(NCORES)))
    LAST_RESULTS = kr
    res = kr.results
    z = np.concatenate([res[c]["z"] for c in range(NCORES)], axis=0).reshape(B, N, D)
    ids = np.concatenate(
        [res[c]["ids"].reshape(-1) for c in range(NCORES)]
    ).reshape(B, N).astype(np.int32)
    total = np.sum([np.float64(res[c]["partial"][0, 0]) for c in range(NCORES)])
    vq_total = np.float32((1.0 + BETA) * total / (B * N * d))
    return z, ids, vq_total
